# revision 71
# baseline (speedup 1.0000x reference)
"""Causal self-attention (B=4, T=2048, C=1024, H=16, D=64) on 8 trn2 cores.

Sharding: core c handles batch b = c//2 and head-group hg = c%2 (8 heads).
The final 2-way partial-sum + bias happens on host.

Per-core program (v3 — flipped attn@v):
  - qkv projections run as fp8e4m3 DoubleRow GEMMs (2-term q/k, 3-term v).
  - q,k are pre-scaled by sqrt(0.125*log2(e)) at evacuation so the scores
    PSUM directly holds y with e = 2^y (exp base 2): ACT uses Exp with
    scale=ln2; optionally some tiles run a 2-instruction custom-DVE exp
    (magic-constant rounding + int32 bitcast pun + quadratic mantissa fix).
  - attn@v is FLIPPED: out = [128 queries, 65(64 v + denominator)] PSUM
    accumulation chains over key tiles (streams 65/matmul instead of 512),
    normalize = per-partition reciprocal * tensor_scalar at evacuation,
    and the [q,f]->[f,q] transpose rides the idle DMA xbar.
  - k bias is dropped (cancels in softmax); v bias folded into b_out on
    host; q bias applied at evacuation time (pre-scaled).
  - evacuations on DVE; tri-mask multiplies on gpsimd (SBUF-only engine).
"""

import os
import sys

for _p in ("/opt/trn_rl_repo", "/root/.axon_site/_ro/trn_rl_repo"):
    if os.path.isdir(_p) and _p not in sys.path:
        sys.path.insert(0, _p)

import numpy as np
import ml_dtypes

B, T, C = 4, 2048, 1024
H, D = 16, 64
NCORES = 8
HPC = 8          # heads per core
FQ = HPC * D     # 512 per-core q (=k=v) feature count
TK = T // 128    # 16 token tiles of 128
V2W = 132        # v2 per-ktile width: (64 v + 1 one + 1 pad) * 2 sides

F8 = ml_dtypes.float8_e4m3
BF = ml_dtypes.bfloat16

QA = float(np.sqrt(0.125 * np.log2(np.e)))   # folded into q AND k scales
LN2 = float(np.log(2.0))
MAGIC = 12582912.0                            # 1.5 * 2^23

# quadratic minimax fit of c2*((f+a)^2+b) ~ 2^f on [-0.5, 0.5]
EXP_A = 1.4751975556380126
EXP_B = 2.0199598192442028
EXP_C2 = 0.238418101744534

_CACHE = {}


def _fit_c1():
    e = int(np.floor(np.log2(EXP_C2)))
    m = EXP_C2 / (2.0 ** e)
    return float(127 + e + (m - 1.0))


def _register_exp_ops():
    """Register the 2-instruction DVE exp (idempotent)."""
    if "ops" in _CACHE:
        return _CACHE["ops"]
    from concourse.dve_ops import (DveOp, OPS, CUSTOM_DVE_SPECS,
                                   _SUB_OPCODE_FOR_NAME)
    from concourse.dve_spec import Spec, Src0, Src1, C0, C1, C2, lower
    from concourse.dve_uop import DveOpSpec

    _t = Src0 + C0
    _i = _t - C0
    _bodyA = (_i + C1) * C2

    def _refA(in0, in1, s0, s1, imm2):
        t = (in0.astype(np.float32) + np.float32(s0)).astype(np.float32)
        i = (t - np.float32(s0)).astype(np.float32)
        u = (i + np.float32(s1)).astype(np.float32)
        return (u * np.float32(imm2)).astype(np.float32)

    _tb = Src1 + C0
    _ib = _tb - C0
    _fb = Src1 - _ib
    _ub = _fb + C1
    _bodyB = Src0 * (_ub * _ub + C2)

    def _refB(in0, in1, s0, s1, imm2):
        y = in1.astype(np.float32)
        t = (y + np.float32(s0)).astype(np.float32)
        i = (t - np.float32(s0)).astype(np.float32)
        f = (y - i).astype(np.float32)
        u = (f + np.float32(s1)).astype(np.float32)
        return (in0.astype(np.float32)
                * (u * u + np.float32(imm2))).astype(np.float32)

    ops = {}
    for name, body, ref in (("ANT_EXPA", _bodyA, _refA),
                            ("ANT_EXPB", _bodyB, _refB)):
        if name in _SUB_OPCODE_FOR_NAME:
            ops[name] = next(o for o in OPS if o.name == name)
            continue
        spec = Spec(body=body, reference=ref)
        tmp = DveOpSpec(name=name, opcode=1, uops=lower(spec, ver="v3"),
                        rd1_en=(name == "ANT_EXPB"))
        op = DveOp(name, spec, subdim=False, uops_sha={"v3": tmp.sha("v3")})
        OPS.append(op)
        CUSTOM_DVE_SPECS[name] = spec
        _SUB_OPCODE_FOR_NAME[name] = 1 + len(_SUB_OPCODE_FOR_NAME)
        ops[name] = op
    _CACHE["ops"] = ops
    return ops


def _build_program():
    import concourse.bacc as bacc
    import concourse.tile as tile
    import concourse.mybir as mybir
    from contextlib import ExitStack

    f32 = mybir.dt.float32
    bf16 = mybir.dt.bfloat16
    fp8 = mybir.dt.float8e4
    i32 = mybir.dt.int32
    AF = mybir.ActivationFunctionType
    ALU = mybir.AluOpType
    DR = mybir.MatmulPerfMode.DoubleRow

    K_XN = int(os.environ.get("K_XN", "0"))
    ops = _register_exp_ops() if K_XN else None

    nc = bacc.Bacc("TRN2", target_bir_lowering=False, debug=False)

    # all inputs are partition-major [128, ...]; x is chunked [c,s,t] so each
    # 512-token chunk is one contiguous run per partition (128 descriptors)
    xhi_d = nc.dram_tensor("x8hi", [128, 8 * T], fp8, kind="ExternalInput").ap()
    xlo_d = nc.dram_tensor("x8lo", [128, 8 * T], fp8, kind="ExternalInput").ap()
    wq0_d = nc.dram_tensor("wq0", [128, 8 * 128], fp8, kind="ExternalInput").ap()
    wk0_d = nc.dram_tensor("wk0", [128, 8 * 128], fp8, kind="ExternalInput").ap()
    wrest_d = nc.dram_tensor("wrest", [128, 8 * 768], fp8,
                             kind="ExternalInput").ap()
    wvh_d = nc.dram_tensor("wv8hi", [128, 8 * FQ], fp8,
                           kind="ExternalInput").ap()
    wvl_d = nc.dram_tensor("wv8lo", [128, 8 * FQ], fp8,
                           kind="ExternalInput").ap()
    wo_d = nc.dram_tensor("wo16", [128, 4 * C], bf16,
                          kind="ExternalInput").ap()
    bq_d = nc.dram_tensor("bq64", [64, 8], f32, kind="ExternalInput").ap()
    tri_d = nc.dram_tensor("tri16", [128, 128], bf16, kind="ExternalInput").ap()
    y_d = nc.dram_tensor("y", [T, C], f32, kind="ExternalOutput").ap()
    dbg = os.environ.get("K_DEBUG", "0") == "1"
    if dbg:
        qdbg_d = nc.dram_tensor("qdbg", [64, 2 * T], fp8,
                                kind="ExternalOutput").ap()
        kdbg_d = nc.dram_tensor("kdbg", [64, 2 * T], fp8,
                                kind="ExternalOutput").ap()
        vdbg_d = nc.dram_tensor("vdbg", [128, 4 * TK * V2W], bf16,
                                kind="ExternalOutput").ap()
        cdbg_d = nc.dram_tensor("cdbg", [128, T], bf16,
                                kind="ExternalOutput").ap()
        edbg_d = nc.dram_tensor("edbg", [128, 1024], bf16,
                                kind="ExternalOutput").ap()
        ndbg_d = nc.dram_tensor("ndbg", [128, 4 * 128], bf16,
                                kind="ExternalOutput").ap()
        rdbg_d = nc.dram_tensor("rdbg", [128, 8], f32,
                                kind="ExternalOutput").ap()
        adbg_d = nc.dram_tensor("adbg", [128, 1024], f32,
                                kind="ExternalOutput").ap()

    with tile.TileContext(nc) as tc, ExitStack() as ctx:
        pp = ctx.enter_context(tc.tile_pool(name="persist", bufs=1))
        x_hi = pp.tile([128, 8 * T], fp8, tag="xhi", name="x_hi")
        x_lo = pp.tile([128, 8 * T], fp8, tag="xlo", name="x_lo")
        wq0_t = pp.tile([128, 8 * 128], fp8, tag="wq0", name="wq0_t")
        wk0_t = pp.tile([128, 8 * 128], fp8, tag="wk0", name="wk0_t")
        wrest_t = pp.tile([128, 8 * 768], fp8, tag="wre", name="wrest_t")
        wv_hi = pp.tile([128, 8 * FQ], fp8, tag="wvh", name="wv_hi")
        wv_lo = pp.tile([128, 8 * FQ], fp8, tag="wvl", name="wv_lo")
        wo_sb = pp.tile([128, 4 * C], bf16, tag="wo", name="wo_sb")
        bq_sb = pp.tile([64, 8], f32, tag="bq", name="bq_sb")
        tri_sb = pp.tile([128, 128], bf16, tag="tri", name="tri_sb")
        qT = [pp.tile([64, 2 * T], fp8, tag=f"q{p}", name=f"qT{p}")
              for p in range(4)]
        kT = [pp.tile([64, 2 * T], fp8, tag=f"k{p}", name=f"kT{p}")
              for p in range(4)]
        v2all = pp.tile([128, 4 * TK * V2W], bf16, tag="v2", name="v2all")
        ctx4 = [pp.tile([128, T], bf16, tag=f"c{p}", name=f"ctx4_{p}")
                for p in range(4)]

        # x chunked: [p, chunk(4), slot(8), token(512)]
        xhi_c = x_hi.rearrange("p (c s t) -> p c s t", c=4, s=8)
        xlo_c = x_lo.rearrange("p (c s t) -> p c s t", c=4, s=8)
        xhi_dc = xhi_d.rearrange("p (c f) -> p c f", c=4)
        xlo_dc = xlo_d.rearrange("p (c f) -> p c f", c=4)
        xhi_f = x_hi.rearrange("p (c f) -> p c f", c=4)
        xlo_f = x_lo.rearrange("p (c f) -> p c f", c=4)
        wq0_v = wq0_t.rearrange("p (s f) -> p s f", s=8)
        wk0_v = wk0_t.rearrange("p (s f) -> p s f", s=8)
        wrest_v = wrest_t.rearrange("p (s f) -> p s f", s=8)
        wvh_v = wv_hi.rearrange("p (s f) -> p s f", s=8)
        wvl_v = wv_lo.rearrange("p (s f) -> p s f", s=8)
        qT_v = [t.rearrange("p (s t) -> p s t", s=2) for t in qT]
        kT_v = [t.rearrange("p (s t) -> p s t", s=2) for t in kT]
        v2_v = v2all.rearrange("p (pr k sd w) -> p pr k sd w", pr=4, k=TK,
                               sd=2)

        # int32/f32 punned arenas for the DVE exp (raw bass allocs)
        zpairs = []
        if K_XN:
            for zi in range(3):
                zt = nc.alloc_sbuf_tensor(f"zint{zi}", [128, 1024], i32)
                zaddr = nc.lookup_mloc(zt).addr
                gp = nc.alloc_sbuf_tensor_at(
                    f"zpun{zi}", [128, 1024], f32, offset=zaddr)
                zpairs.append((zt.ap(), gp.ap()))

        # critical-path loads first: q/k pair-0 weights (scalar q) + x chunk 0
        # (sync q) + x_lo chunk 0 (swdge); bulk weights ride the DVE queue so
        # the ACT sequencer stays free for the first exps
        nc.scalar.dma_start(out=wq0_t, in_=wq0_d)
        nc.scalar.dma_start(out=wk0_t, in_=wk0_d)
        nc.sync.dma_start(out=xhi_f[:, 0, :], in_=xhi_dc[:, 0, :])
        nc.gpsimd.dma_start(out=xlo_f[:, 0, :], in_=xlo_dc[:, 0, :])
        nc.sync.dma_start(out=bq_sb, in_=bq_d)
        nc.scalar.dma_start(out=tri_sb, in_=tri_d)
        nc.gpsimd.dma_start(out=wv_hi, in_=wvh_d)
        nc.gpsimd.dma_start(out=wv_lo, in_=wvl_d)
        for c in range(1, 4):
            nc.sync.dma_start(out=xhi_f[:, c, :], in_=xhi_dc[:, c, :])
            nc.gpsimd.dma_start(out=xlo_f[:, c, :], in_=xlo_dc[:, c, :])
        nc.scalar.dma_start(out=wrest_t, in_=wrest_d)
        nc.scalar.dma_start(out=wo_sb, in_=wo_d)

        # ones columns of v2 (softmax denominator rows)
        nc.vector.memset(v2_v[:, :, :, 0:1, 64:65], 1.0)
        nc.vector.memset(v2_v[:, :, :, 1:2, 64:65], 1.0)
        # warm the ACT Exp table during the initial DMA wait
        warm = pp.tile([1, 8], bf16, tag="warm", name="warm")
        nc.scalar.activation(warm, bq_sb[0:1, 0:8], AF.Exp, scale=1.0)

        qkps = ctx.enter_context(
            tc.tile_pool(name="qkps", bufs=2, space="PSUM"))
        scps = ctx.enter_context(
            tc.tile_pool(name="scps", bufs=2, space="PSUM"))
        avps = ctx.enter_context(
            tc.tile_pool(name="avps", bufs=2, space="PSUM"))
        esb = ctx.enter_context(
            tc.tile_pool(name="esb",
                         bufs=int(os.environ.get("K_LAG", "4"))
                         + int(os.environ.get("K_XOVER", "2")) + 2))
        coll = ctx.enter_context(tc.tile_pool(name="coll", bufs=4))
        nrmp = ctx.enter_context(tc.tile_pool(name="nrmp", bufs=12))
        ystg = ctx.enter_context(tc.tile_pool(name="ystg", bufs=8))

        xstate = {"n": 0, "z": 0}
        dbg_e = []

        def dve_exp(e, scp):
            z, g = zpairs[xstate["z"]]
            xstate["z"] = (xstate["z"] + 1) % len(zpairs)
            nc.vector._custom_dve(ops["ANT_EXPA"], out=z, in0=scp,
                                  s0=MAGIC, s1=_fit_c1(),
                                  imm2=float(2 ** 23))
            nc.vector._custom_dve(ops["ANT_EXPB"], out=e, in0=g, in1=scp,
                                  s0=MAGIC, s1=EXP_A, imm2=EXP_B)

        def emit_qk(tau, pair, c, eng=None):
            """q or k projection for one pair's 2 heads, one token chunk."""
            if pair == 0:
                wt, wcol = (wq0_v if tau == 0 else wk0_v), 0
            else:
                wt, wcol = wrest_v, (tau * 3 + pair - 1) * 128
            ps = qkps.tile([128, 512], f32, tag="p512", name="qk_ps")
            ti = 0
            for xt in (xhi_c, xlo_c):
                for ck in range(4):
                    nc.tensor.matmul(
                        ps,
                        lhsT=wt[:, 2 * ck:2 * ck + 2, wcol:wcol + 128],
                        rhs=xt[:, c, 2 * ck:2 * ck + 2, :],
                        start=(ti == 0), stop=(ti == 7), perf_mode=DR)
                    ti += 1
            dest = qT[pair] if tau == 0 else kT[pair]
            for s in range(2):
                dst = dest[:, s * T + c * 512: s * T + (c + 1) * 512]
                if tau == 0:
                    nc.vector.tensor_scalar(
                        out=dst, in0=ps[64 * s:64 * s + 64, :],
                        scalar1=QA / 64.0,
                        scalar2=bq_sb[:, pair * 2 + s:pair * 2 + s + 1],
                        op0=ALU.mult, op1=ALU.add)
                else:
                    nc.vector.tensor_scalar_mul(dst, ps[64 * s:64 * s + 64, :],
                                                QA / 64.0)

        def emit_v(kt, half, eng=None):
            """v for 2 pairs' 4 heads, one key tile, [keys, feat] layout."""
            eng = eng or nc.vector
            ps = qkps.tile([128, 512], f32, tag="p512", name="v_ps")
            ti = 0
            cc, tk0 = kt // 4, (kt % 4) * 128
            for xt, wt in ((xhi_c, wvh_v), (xhi_c, wvl_v), (xlo_c, wvh_v)):
                for ck in range(4):
                    nc.tensor.matmul(
                        ps[:, 0:256],
                        lhsT=xt[:, cc, 2 * ck:2 * ck + 2, tk0:tk0 + 128],
                        rhs=wt[:, 2 * ck:2 * ck + 2,
                               half * 256:(half + 1) * 256],
                        start=(ti == 0), stop=(ti == 11), perf_mode=DR)
                    ti += 1
            psr = ps.rearrange("p (pr sd f) -> p pr sd f", pr=4, sd=2)
            eng.tensor_scalar_mul(
                v2_v[:, 2 * half:2 * half + 2, kt:kt + 1, :, 0:64],
                psr[:, 0:2, :, :], 1.0 / 64.0)

        # ---- filler machinery (see baseline): interleave qkv/outproj ----
        filler = []
        fill_state = {"emitted": 0.0, "groups": 0}
        FILL_PER_GROUP = float(os.environ.get("K_FPG", "450"))
        K_BOOST = float(os.environ.get("K_BOOST", "2600"))
        K_F0 = int(os.environ.get("K_F0", "1"))
        K_F123 = int(os.environ.get("K_F123", "0"))
        K_NB = int(os.environ.get("K_NB", "4"))
        K_LAG = int(os.environ.get("K_LAG", "4"))
        K_XOVER = int(os.environ.get("K_XOVER", "0"))

        def pump(force=0, boost=0.0):
            fill_state["groups"] += 1
            fill_state["bonus"] = fill_state.get("bonus", 0.0) + boost
            budget = (fill_state["groups"] * FILL_PER_GROUP
                      + fill_state["bonus"])
            popped = 0
            # deadline-due units first (≤2 per pump) so they spread across
            # the preceding attn instead of bursting at the boundary
            force = max(force,
                        sum(1 for u in filler[:4]
                            if u[0] <= fill_state.get("cur", 0)))
            while filler:
                if popped < force:
                    pass
                elif (fill_state["emitted"] >= budget
                      or fill_state["groups"] < filler[0][1]):
                    break
                _, _, cost, fn = filler.pop(0)
                fn()
                fill_state["emitted"] += cost
                popped += 1

        pend = []   # av batches lagged K_LAG group-sides behind exp,
        # across attn boundaries (each entry is a bound thunk)

        def attn(pair, qc, force=0, boost=0.0):
            ngr = 2 * (qc + 1)
            av = [avps.tile([128, 512], f32, tag="av", name="av_ps")
                  for _ in range(2)]
            rc = [coll.tile([128, 4], f32, tag="rc", name="recip4")
                  for _ in range(2)]
            nrm = [nrmp.tile([128, 128], bf16, tag="nr", name="nrm")
                   for _ in range(4)]

            # K_DIAG1: diag groups first (chains stop on old exps) — measured
            # slower than ascending order, kept as an experiment knob
            if os.environ.get("K_DIAG1", "0") == "1":
                order = list(range(ngr - 2, ngr)) + list(range(ngr - 2))
            else:
                order = list(range(ngr))
            last = {}           # qs -> (g, j) of its final accumulation
            first = {}          # qs -> (g, j) of its first accumulation
            for g in order:
                for j in range(2):
                    kt = 2 * g + j
                    for qs in range(max(0, kt - 4 * qc), 4):
                        last[qs] = (g, j)
                        if qs not in first:
                            first[qs] = (g, j)
            arm = (order[0], 0)         # the very first av batch arms

            sdone = {}

            def finish(side, fins):
                # chains `fins` of this side just stopped: normalize now
                # (releases the av bytes early for the next attn's arming)
                lo, hi = min(fins), max(fins)
                av_q = av[side].rearrange("p (q c) -> p q c", q=4)
                rcv = rc[side].rearrange("p (q o) -> p q o", o=1)
                nc.vector.reciprocal(rcv[:, lo:hi + 1, :],
                                     av_q[:, lo:hi + 1, 64:65])
                for qs in fins:
                    nc.vector.tensor_scalar(
                        out=nrm[qs][:, side * 64:side * 64 + 64],
                        in0=av[side][:, qs * 128:qs * 128 + 64],
                        scalar1=rc[side][:, qs:qs + 1], scalar2=None,
                        op0=ALU.mult)
                    sdone[qs] = sdone.get(qs, 0) + 1
                    if sdone[qs] == 2:
                        qt = qc * 4 + qs
                        nc.sync.dma_start(
                            out=ctx4[pair][:, qt * 128:(qt + 1) * 128],
                            in_=nrm[qs], transpose=True)

            def emit_av(g, side, e):
                # one start=True per av bank: arming marks the whole 2KB
                # zero region pending, so later chains' first writes
                # (start=False) land fresh; a second start would re-arm
                # and wipe siblings' partials.
                for j in range(2):
                    kt = 2 * g + j
                    vb = (pair * TK + kt) * V2W + side * 66
                    for qs in range(max(0, kt - 4 * qc), 4):
                        nc.tensor.matmul(
                            av[side][:, qs * 128:qs * 128 + 65],
                            lhsT=e[:, j * 512 + qs * 128:
                                   j * 512 + (qs + 1) * 128],
                            rhs=v2all[:, vb:vb + 65],
                            start=((g, j) == arm and qs == 0),
                            stop=(last[qs] == (g, j)),
                            skip_group_check=True)
                fins = [qs for qs in range(4) if last[qs][0] == g]
                if fins:
                    finish(side, fins)

            for g in order:
                diag = g >= ngr - 2
                m = g - (ngr - 2)
                for side in range(2):
                    scp = scps.tile([128, 1024], f32, tag="sc", name="sc_ps")
                    for j in range(2):
                        kt = 2 * g + j
                        roff = min((2 * m + j) * 128, 256) if diag else 0
                        nc.tensor.matmul(
                            scp[:, j * 512 + roff:(j + 1) * 512],
                            lhsT=kT_v[pair][32 * side:32 * side + 32, :,
                                            kt * 128:(kt + 1) * 128],
                            rhs=qT_v[pair][32 * side:32 * side + 32, :,
                                           qc * 512 + roff:(qc + 1) * 512],
                            start=True, stop=True, perf_mode=DR)
                    e = esb.tile([128, 1024], bf16, tag="e", name="e_sb")
                    if diag and m == 1:
                        nc.scalar.activation(e[:, 256:512], scp[:, 256:512],
                                             AF.Exp, scale=LN2)
                        nc.scalar.activation(e[:, 896:1024], scp[:, 896:1024],
                                             AF.Exp, scale=LN2)
                    else:
                        xstate["n"] += 1
                        if K_XN and xstate["n"] % K_XN == 0:
                            dve_exp(e, scp)
                        else:
                            nc.scalar.activation(e, scp, AF.Exp, scale=LN2)
                    if diag:
                        for j in range(2):
                            r = 2 * m + j
                            c0 = j * 512 + r * 128
                            nc.vector.tensor_mul(
                                e[:, c0:c0 + 128], e[:, c0:c0 + 128], tri_sb)
                    # lagged av batches stall on nothing (their exps are old),
                    # so emit them first; fillers follow (keeps finish-norms
                    # ahead of filler evacs in the DVE queue)
                    pend.append(lambda g=g, s=side, ee=e, f=emit_av: f(g, s, ee))
                    if len(pend) > K_LAG:
                        pend.pop(0)()
                    pump(force=force if side == 0 else 0, boost=boost)
                    if dbg and pair == 0 and qc == 0 and g == 0 and side == 0:
                        nc.sync.dma_start(out=edbg_d, in_=e)
            while len(pend) > K_XOVER:
                pend.pop(0)()

        def outproj(tt, oc, tail=False):
            # tail units alternate onto the (now idle) avps bank ring so two
            # units pipeline instead of serializing on one psum ring
            pool = avps if (tail and (tt + oc) % 2 == 0) else qkps
            tag = "av" if pool is avps else "p512"
            yp = pool.tile([128, 512], f32, tag=tag, name="y_ps")
            for f in range(4):
                nc.tensor.matmul(
                    yp, lhsT=ctx4[f][:, tt * 128:(tt + 1) * 128],
                    rhs=wo_sb[:, f * C + oc * 512: f * C + (oc + 1) * 512],
                    start=(f == 0), stop=(f == 3))
            ys = ystg.tile([128, 512], f32, tag="ys", name="y_sb")
            if tail and (tt + oc) % 2 == 1:
                # ACT engine and its DGE queue are idle in the tail
                nc.scalar.activation(ys, yp, AF.Identity, scale=1.0)
                nc.scalar.dma_start(
                    out=y_d[tt * 128:(tt + 1) * 128, oc * 512:(oc + 1) * 512],
                    in_=ys)
            else:
                nc.vector.tensor_copy(ys, yp)
                nc.gpsimd.dma_start(
                    out=y_d[tt * 128:(tt + 1) * 128, oc * 512:(oc + 1) * 512],
                    in_=ys)

        # ---- emission schedule (program order = scheduler priority) ----
        if os.environ.get("K_P3D", "0") == "1":
            # pair 3 descending: outproj(qc3) releases early, program ends
            # on the smallest attention
            SEQ = [(p, qc) for p in range(3) for qc in range(4)] + \
                [(3, qc) for qc in (3, 2, 1, 0)]
        else:
            SEQ = [(p, qc) for p in range(4) for qc in range(4)]
        IDX = {pq: i for i, pq in enumerate(SEQ)}

        def unit(dl, cost, fn, nb=0):
            filler.append((dl, nb, cost, fn))

        def flush_until(idx):
            while filler and filler[0][0] <= idx:
                _, _, cost, fn = filler.pop(0)
                fn()
                fill_state["emitted"] += cost

        emit_qk(0, 0, 0, eng=nc.vector)
        emit_qk(1, 0, 0, eng=nc.vector)

        def qk_unit(tau, pair, c):
            unit(max(IDX[(pair, c)] - 1, 0), 854.0,
                 lambda t=tau, p=pair, cc=c: emit_qk(t, p, cc))

        def v_unit(kt, half):
            dl = IDX[(0 if half == 0 else 2, kt // 4)] - 1
            unit(max(dl, 1), 640.0, lambda k=kt, h=half: emit_v(k, h))

        for kt in range(4):
            v_unit(kt, 0)
        qk_unit(0, 0, 1)
        qk_unit(1, 0, 1)
        for kt in range(4, 8):
            v_unit(kt, 0)
        for c in range(2, 4):
            qk_unit(0, 0, c)
            qk_unit(1, 0, c)
            for kt in range(4 * c, 4 * c + 4):
                v_unit(kt, 0)
        for pair in range(1, 4):
            crange = ((3, 2, 1, 0) if pair == 3
                      and os.environ.get("K_P3D", "0") == "1" else range(4))
            for c in crange:
                qk_unit(0, pair, c)
                qk_unit(1, pair, c)
                if pair == 2:
                    for kt in range(4 * c, 4 * c + 4):
                        v_unit(kt, 1)

        for i, (pair, qc) in enumerate(SEQ):
            fill_state["cur"] = i
            flush_until(i)
            attn(pair, qc,
                 force=(K_F0 if i == 0 else (K_F123 if i <= 3 else 0)),
                 boost=K_BOOST if pair == 3 else 0.0)
            if pair == 3:
                for tt in range(4 * qc, 4 * qc + 4):
                    for oc in range(2):
                        unit(99, 853.0,
                             lambda t=tt, o=oc, tl=(i == 15): outproj(t, o, tl),
                             nb=fill_state["groups"] + K_NB)
        while pend:
            pend.pop(0)()
        while filler:
            _, _, cost, fn = filler.pop(0)
            fn()
        if dbg:
            nc.sync.dma_start(out=qdbg_d, in_=qT[0])
            nc.sync.dma_start(out=kdbg_d, in_=kT[0])
            nc.sync.dma_start(out=vdbg_d, in_=v2all)
            nc.sync.dma_start(out=cdbg_d, in_=ctx4[0])

    nc.compile()
    return nc


def _host_inputs(x, w_qkv, b_qkv, w_out):
    """Build the 8 per-core input maps (all partition-major layouts)."""
    f32 = np.float32
    tri = (np.arange(128)[:, None] <= np.arange(128)[None, :]).astype(BF)

    def split8(a):
        hi = a.astype(F8)
        lo = (a - hi.astype(f32)).astype(F8)
        return hi, lo

    def pmajor(a):
        w = a.shape[1]
        return np.ascontiguousarray(
            a.reshape(8, 128, w).transpose(1, 0, 2).reshape(128, 8 * w))

    def pmajor_x(a):
        # [C=(s p), T] -> [p, (chunk, s, t)]: 512-token chunks contiguous
        return np.ascontiguousarray(
            a.reshape(8, 128, 4, 512).transpose(1, 2, 0, 3).reshape(128, -1))

    xs = []
    for b in range(B):
        hi, lo = split8(np.ascontiguousarray(x[b].T))
        xs.append((pmajor_x(hi.astype(F8)), pmajor_x(lo.astype(F8))))

    p = np.arange(128)
    head_of_p = (p % 64) // 32
    feat_of_p = 32 * (p // 64) + (p % 32)

    in_maps = []
    for core in range(NCORES):
        b, hg = core // 2, core % 2
        cols = np.empty(1024, dtype=np.int64)
        for tau in range(2):
            for pair in range(4):
                base = (tau * 4 + pair) * 128
                cols[base:base + 128] = (tau * C + hg * FQ
                                         + (pair * 2 + head_of_p) * 64
                                         + feat_of_p)
        wqk = (np.ascontiguousarray(w_qkv[:, cols]) * 64.0).astype(F8)
        wq0 = pmajor(wqk[:, 0:128])
        wk0 = pmajor(wqk[:, 512:640])
        wrest = pmajor(np.concatenate(
            [wqk[:, 128:512], wqk[:, 640:1024]], axis=1))
        wv = w_qkv[:, 2 * C + hg * FQ: 2 * C + (hg + 1) * FQ] * 64.0
        wv_hi, wv_lo = split8(np.ascontiguousarray(wv))
        wv_hi, wv_lo = pmajor(wv_hi.astype(F8)), pmajor(wv_lo.astype(F8))

        po = np.arange(128)
        rows = np.empty(FQ, dtype=np.int64)
        for f in range(4):
            rows[f * 128:(f + 1) * 128] = (hg * FQ + (2 * f + po // 64) * 64
                                           + po % 64)
        wo16 = np.ascontiguousarray(
            w_out[rows, :].reshape(4, 128, C).transpose(1, 0, 2)
            .reshape(128, 4 * C)).astype(BF)

        # q bias laid out [64, (pair, slot)], pre-scaled by QA
        p64 = np.arange(64)
        bq64 = np.empty((64, 8), dtype=f32)
        for pair in range(4):
            for s in range(2):
                idx = hg * FQ + (pair * 2 + p64 // 32) * 64 + 32 * s + p64 % 32
                bq64[:, pair * 2 + s] = b_qkv[idx] * QA
        in_maps.append({
            "x8hi": xs[b][0], "x8lo": xs[b][1],
            "wq0": wq0, "wk0": wk0, "wrest": wrest,
            "wv8hi": wv_hi, "wv8lo": wv_lo,
            "wo16": wo16, "bq64": bq64, "tri16": tri,
        })
    return in_maps


def get_program():
    if "nc" not in _CACHE:
        _CACHE["nc"] = _build_program()
    return _CACHE["nc"]


def kernel(x, w_qkv, b_qkv, w_out, b_out):
    from concourse.bass_utils import run_bass_kernel_spmd

    x = np.asarray(x, dtype=np.float32)
    w_qkv = np.asarray(w_qkv, dtype=np.float32)
    b_qkv = np.asarray(b_qkv, dtype=np.float32)
    w_out = np.asarray(w_out, dtype=np.float32)
    b_out = np.asarray(b_out, dtype=np.float32)

    nc = get_program()
    in_maps = _host_inputs(x, w_qkv, b_qkv, w_out)
    res = run_bass_kernel_spmd(nc, in_maps, core_ids=list(range(NCORES)))

    bias = b_out + b_qkv[2 * C:] @ w_out        # folded v-bias
    out = np.empty((B, T, C), dtype=np.float32)
    for b in range(B):
        out[b] = res.results[2 * b]["y"] + res.results[2 * b + 1]["y"] + bias
    return out


# revision 72
# speedup vs baseline: 1.0099x; 1.0099x over previous
"""Causal self-attention (B=4, T=2048, C=1024, H=16, D=64) on 8 trn2 cores.

Sharding: core c handles batch b = c//2 and head-group hg = c%2 (8 heads).
The final 2-way partial-sum + bias happens on host.

Per-core program (v3 — flipped attn@v):
  - qkv projections run as fp8e4m3 DoubleRow GEMMs (2-term q/k, 3-term v).
  - q,k are pre-scaled by sqrt(0.125*log2(e)) at evacuation so the scores
    PSUM directly holds y with e = 2^y (exp base 2): ACT uses Exp with
    scale=ln2; optionally some tiles run a 2-instruction custom-DVE exp
    (magic-constant rounding + int32 bitcast pun + quadratic mantissa fix).
  - attn@v is FLIPPED: out = [128 queries, 65(64 v + denominator)] PSUM
    accumulation chains over key tiles (streams 65/matmul instead of 512),
    normalize = per-partition reciprocal * tensor_scalar at evacuation,
    and the [q,f]->[f,q] transpose rides the idle DMA xbar.
  - k bias is dropped (cancels in softmax); v bias folded into b_out on
    host; q bias applied at evacuation time (pre-scaled).
  - evacuations on DVE; tri-mask multiplies on gpsimd (SBUF-only engine).
"""

import os
import sys

for _p in ("/opt/trn_rl_repo", "/root/.axon_site/_ro/trn_rl_repo"):
    if os.path.isdir(_p) and _p not in sys.path:
        sys.path.insert(0, _p)

import numpy as np
import ml_dtypes

B, T, C = 4, 2048, 1024
H, D = 16, 64
NCORES = 8
HPC = 8          # heads per core
FQ = HPC * D     # 512 per-core q (=k=v) feature count
TK = T // 128    # 16 token tiles of 128
V2W = 132        # v2 per-ktile width: (64 v + 1 one + 1 pad) * 2 sides

F8 = ml_dtypes.float8_e4m3
BF = ml_dtypes.bfloat16

QA = float(np.sqrt(0.125 * np.log2(np.e)))   # folded into q AND k scales
LN2 = float(np.log(2.0))
MAGIC = 12582912.0                            # 1.5 * 2^23

# quadratic minimax fit of c2*((f+a)^2+b) ~ 2^f on [-0.5, 0.5]
EXP_A = 1.4751975556380126
EXP_B = 2.0199598192442028
EXP_C2 = 0.238418101744534

_CACHE = {}


def _fit_c1():
    e = int(np.floor(np.log2(EXP_C2)))
    m = EXP_C2 / (2.0 ** e)
    return float(127 + e + (m - 1.0))


def _register_exp_ops():
    """Register the 2-instruction DVE exp (idempotent)."""
    if "ops" in _CACHE:
        return _CACHE["ops"]
    from concourse.dve_ops import (DveOp, OPS, CUSTOM_DVE_SPECS,
                                   _SUB_OPCODE_FOR_NAME)
    from concourse.dve_spec import Spec, Src0, Src1, C0, C1, C2, lower
    from concourse.dve_uop import DveOpSpec

    _t = Src0 + C0
    _i = _t - C0
    _bodyA = (_i + C1) * C2

    def _refA(in0, in1, s0, s1, imm2):
        t = (in0.astype(np.float32) + np.float32(s0)).astype(np.float32)
        i = (t - np.float32(s0)).astype(np.float32)
        u = (i + np.float32(s1)).astype(np.float32)
        return (u * np.float32(imm2)).astype(np.float32)

    _tb = Src1 + C0
    _ib = _tb - C0
    _fb = Src1 - _ib
    _ub = _fb + C1
    _bodyB = Src0 * (_ub * _ub + C2)

    def _refB(in0, in1, s0, s1, imm2):
        y = in1.astype(np.float32)
        t = (y + np.float32(s0)).astype(np.float32)
        i = (t - np.float32(s0)).astype(np.float32)
        f = (y - i).astype(np.float32)
        u = (f + np.float32(s1)).astype(np.float32)
        return (in0.astype(np.float32)
                * (u * u + np.float32(imm2))).astype(np.float32)

    ops = {}
    for name, body, ref in (("ANT_EXPA", _bodyA, _refA),
                            ("ANT_EXPB", _bodyB, _refB)):
        if name in _SUB_OPCODE_FOR_NAME:
            ops[name] = next(o for o in OPS if o.name == name)
            continue
        spec = Spec(body=body, reference=ref)
        tmp = DveOpSpec(name=name, opcode=1, uops=lower(spec, ver="v3"),
                        rd1_en=(name == "ANT_EXPB"))
        op = DveOp(name, spec, subdim=False, uops_sha={"v3": tmp.sha("v3")})
        OPS.append(op)
        CUSTOM_DVE_SPECS[name] = spec
        _SUB_OPCODE_FOR_NAME[name] = 1 + len(_SUB_OPCODE_FOR_NAME)
        ops[name] = op
    _CACHE["ops"] = ops
    return ops


def _build_program():
    import concourse.bacc as bacc
    import concourse.tile as tile
    import concourse.mybir as mybir
    from contextlib import ExitStack

    f32 = mybir.dt.float32
    bf16 = mybir.dt.bfloat16
    fp8 = mybir.dt.float8e4
    i32 = mybir.dt.int32
    AF = mybir.ActivationFunctionType
    ALU = mybir.AluOpType
    DR = mybir.MatmulPerfMode.DoubleRow

    K_XN = int(os.environ.get("K_XN", "0"))
    ops = _register_exp_ops() if K_XN else None

    nc = bacc.Bacc("TRN2", target_bir_lowering=False, debug=False)

    # all inputs are partition-major [128, ...]; x is chunked [c,s,t] so each
    # 512-token chunk is one contiguous run per partition (128 descriptors)
    xhi_d = nc.dram_tensor("x8hi", [128, 8 * T], fp8, kind="ExternalInput").ap()
    xlo_d = nc.dram_tensor("x8lo", [128, 8 * T], fp8, kind="ExternalInput").ap()
    wq0_d = nc.dram_tensor("wq0", [128, 8 * 128], fp8, kind="ExternalInput").ap()
    wk0_d = nc.dram_tensor("wk0", [128, 8 * 128], fp8, kind="ExternalInput").ap()
    wrest_d = nc.dram_tensor("wrest", [128, 8 * 768], fp8,
                             kind="ExternalInput").ap()
    wvh_d = nc.dram_tensor("wv8hi", [128, 8 * FQ], fp8,
                           kind="ExternalInput").ap()
    wvl_d = nc.dram_tensor("wv8lo", [128, 8 * FQ], fp8,
                           kind="ExternalInput").ap()
    wo_d = nc.dram_tensor("wo16", [128, 4 * C], bf16,
                          kind="ExternalInput").ap()
    bq_d = nc.dram_tensor("bq64", [64, 8], f32, kind="ExternalInput").ap()
    tri_d = nc.dram_tensor("tri16", [128, 128], bf16, kind="ExternalInput").ap()
    y_d = nc.dram_tensor("y", [T, C], f32, kind="ExternalOutput").ap()
    dbg = os.environ.get("K_DEBUG", "0") == "1"
    if dbg:
        qdbg_d = nc.dram_tensor("qdbg", [64, 2 * T], fp8,
                                kind="ExternalOutput").ap()
        kdbg_d = nc.dram_tensor("kdbg", [64, 2 * T], fp8,
                                kind="ExternalOutput").ap()
        vdbg_d = nc.dram_tensor("vdbg", [128, 4 * TK * V2W], bf16,
                                kind="ExternalOutput").ap()
        cdbg_d = nc.dram_tensor("cdbg", [128, T], bf16,
                                kind="ExternalOutput").ap()
        edbg_d = nc.dram_tensor("edbg", [128, 1024], bf16,
                                kind="ExternalOutput").ap()
        ndbg_d = nc.dram_tensor("ndbg", [128, 4 * 128], bf16,
                                kind="ExternalOutput").ap()
        rdbg_d = nc.dram_tensor("rdbg", [128, 8], f32,
                                kind="ExternalOutput").ap()
        adbg_d = nc.dram_tensor("adbg", [128, 1024], f32,
                                kind="ExternalOutput").ap()

    with tile.TileContext(nc) as tc, ExitStack() as ctx:
        pp = ctx.enter_context(tc.tile_pool(name="persist", bufs=1))
        x_hi = pp.tile([128, 8 * T], fp8, tag="xhi", name="x_hi")
        x_lo = pp.tile([128, 8 * T], fp8, tag="xlo", name="x_lo")
        wq0_t = pp.tile([128, 8 * 128], fp8, tag="wq0", name="wq0_t")
        wk0_t = pp.tile([128, 8 * 128], fp8, tag="wk0", name="wk0_t")
        wrest_t = pp.tile([128, 8 * 768], fp8, tag="wre", name="wrest_t")
        wv_hi = pp.tile([128, 8 * FQ], fp8, tag="wvh", name="wv_hi")
        wv_lo = pp.tile([128, 8 * FQ], fp8, tag="wvl", name="wv_lo")
        wo_sb = pp.tile([128, 4 * C], bf16, tag="wo", name="wo_sb")
        bq_sb = pp.tile([64, 8], f32, tag="bq", name="bq_sb")
        tri_sb = pp.tile([128, 128], bf16, tag="tri", name="tri_sb")
        qT = [pp.tile([64, 2 * T], fp8, tag=f"q{p}", name=f"qT{p}")
              for p in range(4)]
        kT = [pp.tile([64, 2 * T], fp8, tag=f"k{p}", name=f"kT{p}")
              for p in range(4)]
        v2all = pp.tile([128, 4 * TK * V2W], bf16, tag="v2", name="v2all")
        ctx4 = [pp.tile([128, T], bf16, tag=f"c{p}", name=f"ctx4_{p}")
                for p in range(4)]

        # x chunked: [p, chunk(4), slot(8), token(512)]
        xhi_c = x_hi.rearrange("p (c s t) -> p c s t", c=4, s=8)
        xlo_c = x_lo.rearrange("p (c s t) -> p c s t", c=4, s=8)
        xhi_dc = xhi_d.rearrange("p (c f) -> p c f", c=4)
        xlo_dc = xlo_d.rearrange("p (c f) -> p c f", c=4)
        xhi_f = x_hi.rearrange("p (c f) -> p c f", c=4)
        xlo_f = x_lo.rearrange("p (c f) -> p c f", c=4)
        wq0_v = wq0_t.rearrange("p (s f) -> p s f", s=8)
        wk0_v = wk0_t.rearrange("p (s f) -> p s f", s=8)
        wrest_v = wrest_t.rearrange("p (s f) -> p s f", s=8)
        wvh_v = wv_hi.rearrange("p (s f) -> p s f", s=8)
        wvl_v = wv_lo.rearrange("p (s f) -> p s f", s=8)
        qT_v = [t.rearrange("p (s t) -> p s t", s=2) for t in qT]
        kT_v = [t.rearrange("p (s t) -> p s t", s=2) for t in kT]
        v2_v = v2all.rearrange("p (pr k sd w) -> p pr k sd w", pr=4, k=TK,
                               sd=2)

        # int32/f32 punned arenas for the DVE exp (raw bass allocs)
        zpairs = []
        if K_XN:
            for zi in range(3):
                zt = nc.alloc_sbuf_tensor(f"zint{zi}", [128, 1024], i32)
                zaddr = nc.lookup_mloc(zt).addr
                gp = nc.alloc_sbuf_tensor_at(
                    f"zpun{zi}", [128, 1024], f32, offset=zaddr)
                zpairs.append((zt.ap(), gp.ap()))

        # critical-path loads first: q/k pair-0 weights (scalar q) + x chunk 0
        # (sync q) + x_lo chunk 0 (swdge); bulk weights ride the DVE queue so
        # the ACT sequencer stays free for the first exps
        nc.scalar.dma_start(out=wq0_t, in_=wq0_d)
        nc.scalar.dma_start(out=wk0_t, in_=wk0_d)
        nc.sync.dma_start(out=xhi_f[:, 0, :], in_=xhi_dc[:, 0, :])
        nc.gpsimd.dma_start(out=xlo_f[:, 0, :], in_=xlo_dc[:, 0, :])
        nc.sync.dma_start(out=bq_sb, in_=bq_d)
        nc.scalar.dma_start(out=tri_sb, in_=tri_d)
        nc.gpsimd.dma_start(out=wv_hi, in_=wvh_d)
        nc.gpsimd.dma_start(out=wv_lo, in_=wvl_d)
        for c in range(1, 4):
            nc.sync.dma_start(out=xhi_f[:, c, :], in_=xhi_dc[:, c, :])
            nc.gpsimd.dma_start(out=xlo_f[:, c, :], in_=xlo_dc[:, c, :])
        nc.scalar.dma_start(out=wrest_t, in_=wrest_d)
        nc.scalar.dma_start(out=wo_sb, in_=wo_d)

        # ones columns of v2 (softmax denominator rows)
        nc.vector.memset(v2_v[:, :, :, 0:1, 64:65], 1.0)
        nc.vector.memset(v2_v[:, :, :, 1:2, 64:65], 1.0)
        # warm the ACT Exp table during the initial DMA wait
        warm = pp.tile([1, 8], bf16, tag="warm", name="warm")
        nc.scalar.activation(warm, bq_sb[0:1, 0:8], AF.Exp, scale=1.0)

        qkps = ctx.enter_context(
            tc.tile_pool(name="qkps", bufs=2, space="PSUM"))
        scps = ctx.enter_context(
            tc.tile_pool(name="scps", bufs=2, space="PSUM"))
        avps = ctx.enter_context(
            tc.tile_pool(name="avps", bufs=2, space="PSUM"))
        esb = ctx.enter_context(
            tc.tile_pool(name="esb",
                         bufs=int(os.environ.get("K_LAG", "4"))
                         + int(os.environ.get("K_XOVER", "2")) + 2))
        coll = ctx.enter_context(tc.tile_pool(name="coll", bufs=4))
        nrmp = ctx.enter_context(tc.tile_pool(name="nrmp", bufs=12))
        ystg = ctx.enter_context(tc.tile_pool(name="ystg", bufs=8))

        xstate = {"n": 0, "z": 0}
        dbg_e = []

        def dve_exp(e, scp):
            z, g = zpairs[xstate["z"]]
            xstate["z"] = (xstate["z"] + 1) % len(zpairs)
            nc.vector._custom_dve(ops["ANT_EXPA"], out=z, in0=scp,
                                  s0=MAGIC, s1=_fit_c1(),
                                  imm2=float(2 ** 23))
            nc.vector._custom_dve(ops["ANT_EXPB"], out=e, in0=g, in1=scp,
                                  s0=MAGIC, s1=EXP_A, imm2=EXP_B)

        def emit_qk(tau, pair, c, eng=None):
            """q or k projection for one pair's 2 heads, one token chunk."""
            if pair == 0:
                wt, wcol = (wq0_v if tau == 0 else wk0_v), 0
            else:
                wt, wcol = wrest_v, (tau * 3 + pair - 1) * 128
            ps = qkps.tile([128, 512], f32, tag="p512", name="qk_ps")
            ti = 0
            for xt in (xhi_c, xlo_c):
                for ck in range(4):
                    nc.tensor.matmul(
                        ps,
                        lhsT=wt[:, 2 * ck:2 * ck + 2, wcol:wcol + 128],
                        rhs=xt[:, c, 2 * ck:2 * ck + 2, :],
                        start=(ti == 0), stop=(ti == 7), perf_mode=DR)
                    ti += 1
            dest = qT[pair] if tau == 0 else kT[pair]
            for s in range(2):
                dst = dest[:, s * T + c * 512: s * T + (c + 1) * 512]
                if tau == 0:
                    nc.vector.tensor_scalar(
                        out=dst, in0=ps[64 * s:64 * s + 64, :],
                        scalar1=QA / 64.0,
                        scalar2=bq_sb[:, pair * 2 + s:pair * 2 + s + 1],
                        op0=ALU.mult, op1=ALU.add)
                else:
                    nc.vector.tensor_scalar_mul(dst, ps[64 * s:64 * s + 64, :],
                                                QA / 64.0)

        def emit_v(kt, half, eng=None):
            """v for 2 pairs' 4 heads, one key tile, [keys, feat] layout."""
            eng = eng or nc.vector
            ps = qkps.tile([128, 512], f32, tag="p512", name="v_ps")
            ti = 0
            cc, tk0 = kt // 4, (kt % 4) * 128
            for xt, wt in ((xhi_c, wvh_v), (xhi_c, wvl_v), (xlo_c, wvh_v)):
                for ck in range(4):
                    nc.tensor.matmul(
                        ps[:, 0:256],
                        lhsT=xt[:, cc, 2 * ck:2 * ck + 2, tk0:tk0 + 128],
                        rhs=wt[:, 2 * ck:2 * ck + 2,
                               half * 256:(half + 1) * 256],
                        start=(ti == 0), stop=(ti == 11), perf_mode=DR)
                    ti += 1
            psr = ps.rearrange("p (pr sd f) -> p pr sd f", pr=4, sd=2)
            eng.tensor_scalar_mul(
                v2_v[:, 2 * half:2 * half + 2, kt:kt + 1, :, 0:64],
                psr[:, 0:2, :, :], 1.0 / 64.0)

        # ---- filler machinery (see baseline): interleave qkv/outproj ----
        filler = []
        fill_state = {"emitted": 0.0, "groups": 0}
        FILL_PER_GROUP = float(os.environ.get("K_FPG", "450"))
        K_BOOST = float(os.environ.get("K_BOOST", "2600"))
        K_F0 = int(os.environ.get("K_F0", "1"))
        K_F123 = int(os.environ.get("K_F123", "0"))
        K_NB = int(os.environ.get("K_NB", "4"))
        K_LAG = int(os.environ.get("K_LAG", "4"))
        K_XOVER = int(os.environ.get("K_XOVER", "0"))

        def pump(force=0, boost=0.0):
            fill_state["groups"] += 1
            fill_state["bonus"] = fill_state.get("bonus", 0.0) + boost
            budget = (fill_state["groups"] * FILL_PER_GROUP
                      + fill_state["bonus"])
            popped = 0
            # deadline-due units first (≤2 per pump) so they spread across
            # the preceding attn instead of bursting at the boundary
            force = max(force,
                        sum(1 for u in filler[:4]
                            if u[0] <= fill_state.get("cur", 0)))
            while filler:
                if popped < force:
                    pass
                elif (fill_state["emitted"] >= budget
                      or fill_state["groups"] < filler[0][1]):
                    break
                _, _, cost, fn = filler.pop(0)
                fn()
                fill_state["emitted"] += cost
                popped += 1

        pend = []   # av batches lagged K_LAG group-sides behind exp,
        # across attn boundaries (each entry is a bound thunk)

        def attn(pair, qc, force=0, boost=0.0):
            ngr = 2 * (qc + 1)
            av = [avps.tile([128, 512], f32, tag="av", name="av_ps")
                  for _ in range(2)]
            rc = [coll.tile([128, 4], f32, tag="rc", name="recip4")
                  for _ in range(2)]
            nrm = [nrmp.tile([128, 128], bf16, tag="nr", name="nrm")
                   for _ in range(4)]

            # K_DIAG1: diag groups first (chains stop on old exps) — measured
            # slower than ascending order, kept as an experiment knob
            if os.environ.get("K_DIAG1", "0") == "1":
                order = list(range(ngr - 2, ngr)) + list(range(ngr - 2))
            else:
                order = list(range(ngr))
            last = {}           # qs -> (g, j) of its final accumulation
            first = {}          # qs -> (g, j) of its first accumulation
            for g in order:
                for j in range(2):
                    kt = 2 * g + j
                    for qs in range(max(0, kt - 4 * qc), 4):
                        last[qs] = (g, j)
                        if qs not in first:
                            first[qs] = (g, j)
            arm = (order[0], 0)         # the very first av batch arms

            sdone = {}

            def finish(side, fins):
                # chains `fins` of this side just stopped: normalize now
                # (releases the av bytes early for the next attn's arming)
                lo, hi = min(fins), max(fins)
                av_q = av[side].rearrange("p (q c) -> p q c", q=4)
                rcv = rc[side].rearrange("p (q o) -> p q o", o=1)
                nc.vector.reciprocal(rcv[:, lo:hi + 1, :],
                                     av_q[:, lo:hi + 1, 64:65])
                for qs in fins:
                    nc.vector.tensor_scalar(
                        out=nrm[qs][:, side * 64:side * 64 + 64],
                        in0=av[side][:, qs * 128:qs * 128 + 64],
                        scalar1=rc[side][:, qs:qs + 1], scalar2=None,
                        op0=ALU.mult)
                    sdone[qs] = sdone.get(qs, 0) + 1
                    if sdone[qs] == 2:
                        qt = qc * 4 + qs
                        nc.sync.dma_start(
                            out=ctx4[pair][:, qt * 128:(qt + 1) * 128],
                            in_=nrm[qs], transpose=True)

            def emit_av(g, side, e):
                # one start=True per av bank: arming marks the whole 2KB
                # zero region pending, so later chains' first writes
                # (start=False) land fresh; a second start would re-arm
                # and wipe siblings' partials.
                for j in range(2):
                    kt = 2 * g + j
                    vb = (pair * TK + kt) * V2W + side * 66
                    for qs in range(max(0, kt - 4 * qc), 4):
                        nc.tensor.matmul(
                            av[side][:, qs * 128:qs * 128 + 65],
                            lhsT=e[:, j * 512 + qs * 128:
                                   j * 512 + (qs + 1) * 128],
                            rhs=v2all[:, vb:vb + 65],
                            start=((g, j) == arm and qs == 0),
                            stop=(last[qs] == (g, j)),
                            skip_group_check=True)
                fins = [qs for qs in range(4) if last[qs][0] == g]
                if fins:
                    finish(side, fins)

            for g in order:
                diag = g >= ngr - 2
                m = g - (ngr - 2)
                for side in range(2):
                    scp = scps.tile([128, 1024], f32, tag="sc", name="sc_ps")
                    for j in range(2):
                        kt = 2 * g + j
                        roff = min((2 * m + j) * 128, 256) if diag else 0
                        nc.tensor.matmul(
                            scp[:, j * 512 + roff:(j + 1) * 512],
                            lhsT=kT_v[pair][32 * side:32 * side + 32, :,
                                            kt * 128:(kt + 1) * 128],
                            rhs=qT_v[pair][32 * side:32 * side + 32, :,
                                           qc * 512 + roff:(qc + 1) * 512],
                            start=True, stop=True, perf_mode=DR)
                    e = esb.tile([128, 1024], bf16, tag="e", name="e_sb")
                    if diag and m == 1:
                        nc.scalar.activation(e[:, 256:512], scp[:, 256:512],
                                             AF.Exp, scale=LN2)
                        nc.scalar.activation(e[:, 896:1024], scp[:, 896:1024],
                                             AF.Exp, scale=LN2)
                    else:
                        xstate["n"] += 1
                        if K_XN and xstate["n"] % K_XN == 0:
                            dve_exp(e, scp)
                        else:
                            nc.scalar.activation(e, scp, AF.Exp, scale=LN2)
                    if diag:
                        for j in range(2):
                            r = 2 * m + j
                            c0 = j * 512 + r * 128
                            nc.gpsimd.tensor_mul(
                                e[:, c0:c0 + 128], e[:, c0:c0 + 128], tri_sb)
                    # lagged av batches stall on nothing (their exps are old),
                    # so emit them first; fillers follow (keeps finish-norms
                    # ahead of filler evacs in the DVE queue)
                    pend.append(lambda g=g, s=side, ee=e, f=emit_av: f(g, s, ee))
                    if len(pend) > K_LAG:
                        pend.pop(0)()
                    pump(force=force if side == 0 else 0, boost=boost)
                    if dbg and pair == 0 and qc == 0 and g == 0 and side == 0:
                        nc.sync.dma_start(out=edbg_d, in_=e)
            while len(pend) > K_XOVER:
                pend.pop(0)()

        def outproj(tt, oc, tail=False):
            # tail units alternate onto the (now idle) avps bank ring so two
            # units pipeline instead of serializing on one psum ring
            pool = avps if (tail and (tt + oc) % 2 == 0) else qkps
            tag = "av" if pool is avps else "p512"
            yp = pool.tile([128, 512], f32, tag=tag, name="y_ps")
            for f in range(4):
                nc.tensor.matmul(
                    yp, lhsT=ctx4[f][:, tt * 128:(tt + 1) * 128],
                    rhs=wo_sb[:, f * C + oc * 512: f * C + (oc + 1) * 512],
                    start=(f == 0), stop=(f == 3))
            ys = ystg.tile([128, 512], f32, tag="ys", name="y_sb")
            if tail and (tt + oc) % 2 == 1:
                # ACT engine and its DGE queue are idle in the tail
                nc.scalar.activation(ys, yp, AF.Identity, scale=1.0)
                nc.scalar.dma_start(
                    out=y_d[tt * 128:(tt + 1) * 128, oc * 512:(oc + 1) * 512],
                    in_=ys)
            else:
                nc.vector.tensor_copy(ys, yp)
                nc.gpsimd.dma_start(
                    out=y_d[tt * 128:(tt + 1) * 128, oc * 512:(oc + 1) * 512],
                    in_=ys)

        # ---- emission schedule (program order = scheduler priority) ----
        if os.environ.get("K_P3D", "0") == "1":
            # pair 3 descending: outproj(qc3) releases early, program ends
            # on the smallest attention
            SEQ = [(p, qc) for p in range(3) for qc in range(4)] + \
                [(3, qc) for qc in (3, 2, 1, 0)]
        else:
            SEQ = [(p, qc) for p in range(4) for qc in range(4)]
        IDX = {pq: i for i, pq in enumerate(SEQ)}

        def unit(dl, cost, fn, nb=0):
            filler.append((dl, nb, cost, fn))

        def flush_until(idx):
            while filler and filler[0][0] <= idx:
                _, _, cost, fn = filler.pop(0)
                fn()
                fill_state["emitted"] += cost

        emit_qk(0, 0, 0, eng=nc.vector)
        emit_qk(1, 0, 0, eng=nc.vector)

        def qk_unit(tau, pair, c):
            unit(max(IDX[(pair, c)] - 1, 0), 854.0,
                 lambda t=tau, p=pair, cc=c: emit_qk(t, p, cc))

        def v_unit(kt, half):
            dl = IDX[(0 if half == 0 else 2, kt // 4)] - 1
            unit(max(dl, 1), 640.0, lambda k=kt, h=half: emit_v(k, h))

        for kt in range(4):
            v_unit(kt, 0)
        qk_unit(0, 0, 1)
        qk_unit(1, 0, 1)
        for kt in range(4, 8):
            v_unit(kt, 0)
        for c in range(2, 4):
            qk_unit(0, 0, c)
            qk_unit(1, 0, c)
            for kt in range(4 * c, 4 * c + 4):
                v_unit(kt, 0)
        for pair in range(1, 4):
            crange = ((3, 2, 1, 0) if pair == 3
                      and os.environ.get("K_P3D", "0") == "1" else range(4))
            for c in crange:
                qk_unit(0, pair, c)
                qk_unit(1, pair, c)
                if pair == 2:
                    for kt in range(4 * c, 4 * c + 4):
                        v_unit(kt, 1)

        for i, (pair, qc) in enumerate(SEQ):
            fill_state["cur"] = i
            flush_until(i)
            attn(pair, qc,
                 force=(K_F0 if i == 0 else (K_F123 if i <= 3 else 0)),
                 boost=K_BOOST if pair == 3 else 0.0)
            if pair == 3:
                for tt in range(4 * qc, 4 * qc + 4):
                    for oc in range(2):
                        unit(99, 853.0,
                             lambda t=tt, o=oc, tl=(i == 15): outproj(t, o, tl),
                             nb=fill_state["groups"] + K_NB)
        while pend:
            pend.pop(0)()
        while filler:
            _, _, cost, fn = filler.pop(0)
            fn()
        if dbg:
            nc.sync.dma_start(out=qdbg_d, in_=qT[0])
            nc.sync.dma_start(out=kdbg_d, in_=kT[0])
            nc.sync.dma_start(out=vdbg_d, in_=v2all)
            nc.sync.dma_start(out=cdbg_d, in_=ctx4[0])

    nc.compile()
    return nc


def _host_inputs(x, w_qkv, b_qkv, w_out):
    """Build the 8 per-core input maps (all partition-major layouts)."""
    f32 = np.float32
    tri = (np.arange(128)[:, None] <= np.arange(128)[None, :]).astype(BF)

    def split8(a):
        hi = a.astype(F8)
        lo = (a - hi.astype(f32)).astype(F8)
        return hi, lo

    def pmajor(a):
        w = a.shape[1]
        return np.ascontiguousarray(
            a.reshape(8, 128, w).transpose(1, 0, 2).reshape(128, 8 * w))

    def pmajor_x(a):
        # [C=(s p), T] -> [p, (chunk, s, t)]: 512-token chunks contiguous
        return np.ascontiguousarray(
            a.reshape(8, 128, 4, 512).transpose(1, 2, 0, 3).reshape(128, -1))

    xs = []
    for b in range(B):
        hi, lo = split8(np.ascontiguousarray(x[b].T))
        xs.append((pmajor_x(hi.astype(F8)), pmajor_x(lo.astype(F8))))

    p = np.arange(128)
    head_of_p = (p % 64) // 32
    feat_of_p = 32 * (p // 64) + (p % 32)

    in_maps = []
    for core in range(NCORES):
        b, hg = core // 2, core % 2
        cols = np.empty(1024, dtype=np.int64)
        for tau in range(2):
            for pair in range(4):
                base = (tau * 4 + pair) * 128
                cols[base:base + 128] = (tau * C + hg * FQ
                                         + (pair * 2 + head_of_p) * 64
                                         + feat_of_p)
        wqk = (np.ascontiguousarray(w_qkv[:, cols]) * 64.0).astype(F8)
        wq0 = pmajor(wqk[:, 0:128])
        wk0 = pmajor(wqk[:, 512:640])
        wrest = pmajor(np.concatenate(
            [wqk[:, 128:512], wqk[:, 640:1024]], axis=1))
        wv = w_qkv[:, 2 * C + hg * FQ: 2 * C + (hg + 1) * FQ] * 64.0
        wv_hi, wv_lo = split8(np.ascontiguousarray(wv))
        wv_hi, wv_lo = pmajor(wv_hi.astype(F8)), pmajor(wv_lo.astype(F8))

        po = np.arange(128)
        rows = np.empty(FQ, dtype=np.int64)
        for f in range(4):
            rows[f * 128:(f + 1) * 128] = (hg * FQ + (2 * f + po // 64) * 64
                                           + po % 64)
        wo16 = np.ascontiguousarray(
            w_out[rows, :].reshape(4, 128, C).transpose(1, 0, 2)
            .reshape(128, 4 * C)).astype(BF)

        # q bias laid out [64, (pair, slot)], pre-scaled by QA
        p64 = np.arange(64)
        bq64 = np.empty((64, 8), dtype=f32)
        for pair in range(4):
            for s in range(2):
                idx = hg * FQ + (pair * 2 + p64 // 32) * 64 + 32 * s + p64 % 32
                bq64[:, pair * 2 + s] = b_qkv[idx] * QA
        in_maps.append({
            "x8hi": xs[b][0], "x8lo": xs[b][1],
            "wq0": wq0, "wk0": wk0, "wrest": wrest,
            "wv8hi": wv_hi, "wv8lo": wv_lo,
            "wo16": wo16, "bq64": bq64, "tri16": tri,
        })
    return in_maps


def get_program():
    if "nc" not in _CACHE:
        _CACHE["nc"] = _build_program()
    return _CACHE["nc"]


def kernel(x, w_qkv, b_qkv, w_out, b_out):
    from concourse.bass_utils import run_bass_kernel_spmd

    x = np.asarray(x, dtype=np.float32)
    w_qkv = np.asarray(w_qkv, dtype=np.float32)
    b_qkv = np.asarray(b_qkv, dtype=np.float32)
    w_out = np.asarray(w_out, dtype=np.float32)
    b_out = np.asarray(b_out, dtype=np.float32)

    nc = get_program()
    in_maps = _host_inputs(x, w_qkv, b_qkv, w_out)
    res = run_bass_kernel_spmd(nc, in_maps, core_ids=list(range(NCORES)))

    bias = b_out + b_qkv[2 * C:] @ w_out        # folded v-bias
    out = np.empty((B, T, C), dtype=np.float32)
    for b in range(B):
        out[b] = res.results[2 * b]["y"] + res.results[2 * b + 1]["y"] + bias
    return out


# revision 73
# speedup vs baseline: 1.0148x; 1.0049x over previous
"""Causal self-attention (B=4, T=2048, C=1024, H=16, D=64) on 8 trn2 cores.

Sharding: core c handles batch b = c//2 and head-group hg = c%2 (8 heads).
The final 2-way partial-sum + bias happens on host.

Per-core program (v3 — flipped attn@v):
  - qkv projections run as fp8e4m3 DoubleRow GEMMs (2-term q/k, 3-term v).
  - q,k are pre-scaled by sqrt(0.125*log2(e)) at evacuation so the scores
    PSUM directly holds y with e = 2^y (exp base 2): ACT uses Exp with
    scale=ln2; optionally some tiles run a 2-instruction custom-DVE exp
    (magic-constant rounding + int32 bitcast pun + quadratic mantissa fix).
  - attn@v is FLIPPED: out = [128 queries, 65(64 v + denominator)] PSUM
    accumulation chains over key tiles (streams 65/matmul instead of 512),
    normalize = per-partition reciprocal * tensor_scalar at evacuation,
    and the [q,f]->[f,q] transpose rides the idle DMA xbar.
  - k bias is dropped (cancels in softmax); v bias folded into b_out on
    host; q bias applied at evacuation time (pre-scaled).
  - evacuations on DVE; tri-mask multiplies on gpsimd (SBUF-only engine).
"""

import os
import sys

for _p in ("/opt/trn_rl_repo", "/root/.axon_site/_ro/trn_rl_repo"):
    if os.path.isdir(_p) and _p not in sys.path:
        sys.path.insert(0, _p)

import numpy as np
import ml_dtypes

B, T, C = 4, 2048, 1024
H, D = 16, 64
NCORES = 8
HPC = 8          # heads per core
FQ = HPC * D     # 512 per-core q (=k=v) feature count
TK = T // 128    # 16 token tiles of 128
V2W = 132        # v2 per-ktile width: (64 v + 1 one + 1 pad) * 2 sides

F8 = ml_dtypes.float8_e4m3
BF = ml_dtypes.bfloat16

QA = float(np.sqrt(0.125 * np.log2(np.e)))   # folded into q AND k scales
LN2 = float(np.log(2.0))
MAGIC = 12582912.0                            # 1.5 * 2^23

# quadratic minimax fit of c2*((f+a)^2+b) ~ 2^f on [-0.5, 0.5]
EXP_A = 1.4751975556380126
EXP_B = 2.0199598192442028
EXP_C2 = 0.238418101744534

_CACHE = {}


def _fit_c1():
    e = int(np.floor(np.log2(EXP_C2)))
    m = EXP_C2 / (2.0 ** e)
    return float(127 + e + (m - 1.0))


def _register_exp_ops():
    """Register the 2-instruction DVE exp (idempotent)."""
    if "ops" in _CACHE:
        return _CACHE["ops"]
    from concourse.dve_ops import (DveOp, OPS, CUSTOM_DVE_SPECS,
                                   _SUB_OPCODE_FOR_NAME)
    from concourse.dve_spec import Spec, Src0, Src1, C0, C1, C2, lower
    from concourse.dve_uop import DveOpSpec

    _t = Src0 + C0
    _i = _t - C0
    _bodyA = (_i + C1) * C2

    def _refA(in0, in1, s0, s1, imm2):
        t = (in0.astype(np.float32) + np.float32(s0)).astype(np.float32)
        i = (t - np.float32(s0)).astype(np.float32)
        u = (i + np.float32(s1)).astype(np.float32)
        return (u * np.float32(imm2)).astype(np.float32)

    _tb = Src1 + C0
    _ib = _tb - C0
    _fb = Src1 - _ib
    _ub = _fb + C1
    _bodyB = Src0 * (_ub * _ub + C2)

    def _refB(in0, in1, s0, s1, imm2):
        y = in1.astype(np.float32)
        t = (y + np.float32(s0)).astype(np.float32)
        i = (t - np.float32(s0)).astype(np.float32)
        f = (y - i).astype(np.float32)
        u = (f + np.float32(s1)).astype(np.float32)
        return (in0.astype(np.float32)
                * (u * u + np.float32(imm2))).astype(np.float32)

    ops = {}
    for name, body, ref in (("ANT_EXPA", _bodyA, _refA),
                            ("ANT_EXPB", _bodyB, _refB)):
        if name in _SUB_OPCODE_FOR_NAME:
            ops[name] = next(o for o in OPS if o.name == name)
            continue
        spec = Spec(body=body, reference=ref)
        tmp = DveOpSpec(name=name, opcode=1, uops=lower(spec, ver="v3"),
                        rd1_en=(name == "ANT_EXPB"))
        op = DveOp(name, spec, subdim=False, uops_sha={"v3": tmp.sha("v3")})
        OPS.append(op)
        CUSTOM_DVE_SPECS[name] = spec
        _SUB_OPCODE_FOR_NAME[name] = 1 + len(_SUB_OPCODE_FOR_NAME)
        ops[name] = op
    _CACHE["ops"] = ops
    return ops


def _build_program():
    import concourse.bacc as bacc
    import concourse.tile as tile
    import concourse.mybir as mybir
    from contextlib import ExitStack

    f32 = mybir.dt.float32
    bf16 = mybir.dt.bfloat16
    fp8 = mybir.dt.float8e4
    i32 = mybir.dt.int32
    AF = mybir.ActivationFunctionType
    ALU = mybir.AluOpType
    DR = mybir.MatmulPerfMode.DoubleRow

    K_XN = int(os.environ.get("K_XN", "0"))
    ops = _register_exp_ops() if K_XN else None

    nc = bacc.Bacc("TRN2", target_bir_lowering=False, debug=False)

    # all inputs are partition-major [128, ...]; x is chunked [c,s,t] so each
    # 512-token chunk is one contiguous run per partition (128 descriptors)
    xhi_d = nc.dram_tensor("x8hi", [128, 8 * T], fp8, kind="ExternalInput").ap()
    xlo_d = nc.dram_tensor("x8lo", [128, 8 * T], fp8, kind="ExternalInput").ap()
    wq0_d = nc.dram_tensor("wq0", [128, 8 * 128], fp8, kind="ExternalInput").ap()
    wk0_d = nc.dram_tensor("wk0", [128, 8 * 128], fp8, kind="ExternalInput").ap()
    wrest_d = nc.dram_tensor("wrest", [128, 8 * 768], fp8,
                             kind="ExternalInput").ap()
    wvh_d = nc.dram_tensor("wv8hi", [128, 8 * FQ], fp8,
                           kind="ExternalInput").ap()
    wvl_d = nc.dram_tensor("wv8lo", [128, 8 * FQ], fp8,
                           kind="ExternalInput").ap()
    wo_d = nc.dram_tensor("wo16", [128, 4 * C], bf16,
                          kind="ExternalInput").ap()
    bq_d = nc.dram_tensor("bq64", [64, 8], f32, kind="ExternalInput").ap()
    tri_d = nc.dram_tensor("tri16", [128, 128], bf16, kind="ExternalInput").ap()
    y_d = nc.dram_tensor("y", [T, C], bf16, kind="ExternalOutput").ap()
    dbg = os.environ.get("K_DEBUG", "0") == "1"
    if dbg:
        qdbg_d = nc.dram_tensor("qdbg", [64, 2 * T], fp8,
                                kind="ExternalOutput").ap()
        kdbg_d = nc.dram_tensor("kdbg", [64, 2 * T], fp8,
                                kind="ExternalOutput").ap()
        vdbg_d = nc.dram_tensor("vdbg", [128, 4 * TK * V2W], bf16,
                                kind="ExternalOutput").ap()
        cdbg_d = nc.dram_tensor("cdbg", [128, T], bf16,
                                kind="ExternalOutput").ap()
        edbg_d = nc.dram_tensor("edbg", [128, 1024], bf16,
                                kind="ExternalOutput").ap()
        ndbg_d = nc.dram_tensor("ndbg", [128, 4 * 128], bf16,
                                kind="ExternalOutput").ap()
        rdbg_d = nc.dram_tensor("rdbg", [128, 8], f32,
                                kind="ExternalOutput").ap()
        adbg_d = nc.dram_tensor("adbg", [128, 1024], f32,
                                kind="ExternalOutput").ap()

    with tile.TileContext(nc) as tc, ExitStack() as ctx:
        pp = ctx.enter_context(tc.tile_pool(name="persist", bufs=1))
        x_hi = pp.tile([128, 8 * T], fp8, tag="xhi", name="x_hi")
        x_lo = pp.tile([128, 8 * T], fp8, tag="xlo", name="x_lo")
        wq0_t = pp.tile([128, 8 * 128], fp8, tag="wq0", name="wq0_t")
        wk0_t = pp.tile([128, 8 * 128], fp8, tag="wk0", name="wk0_t")
        wrest_t = pp.tile([128, 8 * 768], fp8, tag="wre", name="wrest_t")
        wv_hi = pp.tile([128, 8 * FQ], fp8, tag="wvh", name="wv_hi")
        wv_lo = pp.tile([128, 8 * FQ], fp8, tag="wvl", name="wv_lo")
        wo_sb = pp.tile([128, 4 * C], bf16, tag="wo", name="wo_sb")
        bq_sb = pp.tile([64, 8], f32, tag="bq", name="bq_sb")
        tri_sb = pp.tile([128, 128], bf16, tag="tri", name="tri_sb")
        qT = [pp.tile([64, 2 * T], fp8, tag=f"q{p}", name=f"qT{p}")
              for p in range(4)]
        kT = [pp.tile([64, 2 * T], fp8, tag=f"k{p}", name=f"kT{p}")
              for p in range(4)]
        v2all = pp.tile([128, 4 * TK * V2W], bf16, tag="v2", name="v2all")
        ctx4 = [pp.tile([128, T], bf16, tag=f"c{p}", name=f"ctx4_{p}")
                for p in range(4)]

        # x chunked: [p, chunk(4), slot(8), token(512)]
        xhi_c = x_hi.rearrange("p (c s t) -> p c s t", c=4, s=8)
        xlo_c = x_lo.rearrange("p (c s t) -> p c s t", c=4, s=8)
        xhi_dc = xhi_d.rearrange("p (c f) -> p c f", c=4)
        xlo_dc = xlo_d.rearrange("p (c f) -> p c f", c=4)
        xhi_f = x_hi.rearrange("p (c f) -> p c f", c=4)
        xlo_f = x_lo.rearrange("p (c f) -> p c f", c=4)
        wq0_v = wq0_t.rearrange("p (s f) -> p s f", s=8)
        wk0_v = wk0_t.rearrange("p (s f) -> p s f", s=8)
        wrest_v = wrest_t.rearrange("p (s f) -> p s f", s=8)
        wvh_v = wv_hi.rearrange("p (s f) -> p s f", s=8)
        wvl_v = wv_lo.rearrange("p (s f) -> p s f", s=8)
        qT_v = [t.rearrange("p (s t) -> p s t", s=2) for t in qT]
        kT_v = [t.rearrange("p (s t) -> p s t", s=2) for t in kT]
        v2_v = v2all.rearrange("p (pr k sd w) -> p pr k sd w", pr=4, k=TK,
                               sd=2)

        # int32/f32 punned arenas for the DVE exp (raw bass allocs)
        zpairs = []
        if K_XN:
            for zi in range(3):
                zt = nc.alloc_sbuf_tensor(f"zint{zi}", [128, 1024], i32)
                zaddr = nc.lookup_mloc(zt).addr
                gp = nc.alloc_sbuf_tensor_at(
                    f"zpun{zi}", [128, 1024], f32, offset=zaddr)
                zpairs.append((zt.ap(), gp.ap()))

        # critical-path loads first: q/k pair-0 weights (scalar q) + x chunk 0
        # (sync q) + x_lo chunk 0 (swdge); bulk weights ride the DVE queue so
        # the ACT sequencer stays free for the first exps
        nc.scalar.dma_start(out=wq0_t, in_=wq0_d)
        nc.scalar.dma_start(out=wk0_t, in_=wk0_d)
        nc.sync.dma_start(out=xhi_f[:, 0, :], in_=xhi_dc[:, 0, :])
        nc.gpsimd.dma_start(out=xlo_f[:, 0, :], in_=xlo_dc[:, 0, :])
        nc.sync.dma_start(out=bq_sb, in_=bq_d)
        nc.scalar.dma_start(out=tri_sb, in_=tri_d)
        nc.gpsimd.dma_start(out=wv_hi, in_=wvh_d)
        nc.gpsimd.dma_start(out=wv_lo, in_=wvl_d)
        for c in range(1, 4):
            nc.sync.dma_start(out=xhi_f[:, c, :], in_=xhi_dc[:, c, :])
            nc.gpsimd.dma_start(out=xlo_f[:, c, :], in_=xlo_dc[:, c, :])
        nc.scalar.dma_start(out=wrest_t, in_=wrest_d)
        nc.scalar.dma_start(out=wo_sb, in_=wo_d)

        # ones columns of v2 (softmax denominator rows)
        nc.vector.memset(v2_v[:, :, :, 0:1, 64:65], 1.0)
        nc.vector.memset(v2_v[:, :, :, 1:2, 64:65], 1.0)
        # warm the ACT Exp table during the initial DMA wait
        warm = pp.tile([1, 8], bf16, tag="warm", name="warm")
        nc.scalar.activation(warm, bq_sb[0:1, 0:8], AF.Exp, scale=1.0)

        qkps = ctx.enter_context(
            tc.tile_pool(name="qkps", bufs=2, space="PSUM"))
        scps = ctx.enter_context(
            tc.tile_pool(name="scps", bufs=2, space="PSUM"))
        avps = ctx.enter_context(
            tc.tile_pool(name="avps", bufs=2, space="PSUM"))
        esb = ctx.enter_context(
            tc.tile_pool(name="esb",
                         bufs=int(os.environ.get("K_LAG", "4"))
                         + int(os.environ.get("K_XOVER", "2")) + 2))
        coll = ctx.enter_context(tc.tile_pool(name="coll", bufs=4))
        nrmp = ctx.enter_context(tc.tile_pool(name="nrmp", bufs=12))
        ystg = ctx.enter_context(tc.tile_pool(name="ystg", bufs=8))

        xstate = {"n": 0, "z": 0}
        dbg_e = []

        def dve_exp(e, scp):
            z, g = zpairs[xstate["z"]]
            xstate["z"] = (xstate["z"] + 1) % len(zpairs)
            nc.vector._custom_dve(ops["ANT_EXPA"], out=z, in0=scp,
                                  s0=MAGIC, s1=_fit_c1(),
                                  imm2=float(2 ** 23))
            nc.vector._custom_dve(ops["ANT_EXPB"], out=e, in0=g, in1=scp,
                                  s0=MAGIC, s1=EXP_A, imm2=EXP_B)

        def emit_qk(tau, pair, c, eng=None):
            """q or k projection for one pair's 2 heads, one token chunk."""
            if pair == 0:
                wt, wcol = (wq0_v if tau == 0 else wk0_v), 0
            else:
                wt, wcol = wrest_v, (tau * 3 + pair - 1) * 128
            ps = qkps.tile([128, 512], f32, tag="p512", name="qk_ps")
            ti = 0
            for xt in (xhi_c, xlo_c):
                for ck in range(4):
                    nc.tensor.matmul(
                        ps,
                        lhsT=wt[:, 2 * ck:2 * ck + 2, wcol:wcol + 128],
                        rhs=xt[:, c, 2 * ck:2 * ck + 2, :],
                        start=(ti == 0), stop=(ti == 7), perf_mode=DR)
                    ti += 1
            dest = qT[pair] if tau == 0 else kT[pair]
            for s in range(2):
                dst = dest[:, s * T + c * 512: s * T + (c + 1) * 512]
                if tau == 0:
                    nc.vector.tensor_scalar(
                        out=dst, in0=ps[64 * s:64 * s + 64, :],
                        scalar1=QA / 64.0,
                        scalar2=bq_sb[:, pair * 2 + s:pair * 2 + s + 1],
                        op0=ALU.mult, op1=ALU.add)
                else:
                    nc.vector.tensor_scalar_mul(dst, ps[64 * s:64 * s + 64, :],
                                                QA / 64.0)

        def emit_v(kt, half, eng=None):
            """v for 2 pairs' 4 heads, one key tile, [keys, feat] layout."""
            eng = eng or nc.vector
            ps = qkps.tile([128, 512], f32, tag="p512", name="v_ps")
            ti = 0
            cc, tk0 = kt // 4, (kt % 4) * 128
            for xt, wt in ((xhi_c, wvh_v), (xhi_c, wvl_v), (xlo_c, wvh_v)):
                for ck in range(4):
                    nc.tensor.matmul(
                        ps[:, 0:256],
                        lhsT=xt[:, cc, 2 * ck:2 * ck + 2, tk0:tk0 + 128],
                        rhs=wt[:, 2 * ck:2 * ck + 2,
                               half * 256:(half + 1) * 256],
                        start=(ti == 0), stop=(ti == 11), perf_mode=DR)
                    ti += 1
            psr = ps.rearrange("p (pr sd f) -> p pr sd f", pr=4, sd=2)
            eng.tensor_scalar_mul(
                v2_v[:, 2 * half:2 * half + 2, kt:kt + 1, :, 0:64],
                psr[:, 0:2, :, :], 1.0 / 64.0)

        # ---- filler machinery (see baseline): interleave qkv/outproj ----
        filler = []
        fill_state = {"emitted": 0.0, "groups": 0}
        FILL_PER_GROUP = float(os.environ.get("K_FPG", "450"))
        K_BOOST = float(os.environ.get("K_BOOST", "2600"))
        K_F0 = int(os.environ.get("K_F0", "1"))
        K_F123 = int(os.environ.get("K_F123", "0"))
        K_NB = int(os.environ.get("K_NB", "4"))
        K_LAG = int(os.environ.get("K_LAG", "4"))
        K_XOVER = int(os.environ.get("K_XOVER", "0"))

        def pump(force=0, boost=0.0):
            fill_state["groups"] += 1
            fill_state["bonus"] = fill_state.get("bonus", 0.0) + boost
            budget = (fill_state["groups"] * FILL_PER_GROUP
                      + fill_state["bonus"])
            popped = 0
            # deadline-due units first (≤2 per pump) so they spread across
            # the preceding attn instead of bursting at the boundary
            force = max(force,
                        sum(1 for u in filler[:4]
                            if u[0] <= fill_state.get("cur", 0)))
            while filler:
                if popped < force:
                    pass
                elif (fill_state["emitted"] >= budget
                      or fill_state["groups"] < filler[0][1]):
                    break
                _, _, cost, fn = filler.pop(0)
                fn()
                fill_state["emitted"] += cost
                popped += 1

        pend = []   # av batches lagged K_LAG group-sides behind exp,
        # across attn boundaries (each entry is a bound thunk)

        def attn(pair, qc, force=0, boost=0.0):
            ngr = 2 * (qc + 1)
            av = [avps.tile([128, 512], f32, tag="av", name="av_ps")
                  for _ in range(2)]
            rc = [coll.tile([128, 4], f32, tag="rc", name="recip4")
                  for _ in range(2)]
            nrm = [nrmp.tile([128, 128], bf16, tag="nr", name="nrm")
                   for _ in range(4)]

            # K_DIAG1: diag groups first (chains stop on old exps) — measured
            # slower than ascending order, kept as an experiment knob
            if os.environ.get("K_DIAG1", "0") == "1":
                order = list(range(ngr - 2, ngr)) + list(range(ngr - 2))
            else:
                order = list(range(ngr))
            last = {}           # qs -> (g, j) of its final accumulation
            first = {}          # qs -> (g, j) of its first accumulation
            for g in order:
                for j in range(2):
                    kt = 2 * g + j
                    for qs in range(max(0, kt - 4 * qc), 4):
                        last[qs] = (g, j)
                        if qs not in first:
                            first[qs] = (g, j)
            arm = (order[0], 0)         # the very first av batch arms

            sdone = {}

            def finish(side, fins):
                # chains `fins` of this side just stopped: normalize now
                # (releases the av bytes early for the next attn's arming)
                lo, hi = min(fins), max(fins)
                av_q = av[side].rearrange("p (q c) -> p q c", q=4)
                rcv = rc[side].rearrange("p (q o) -> p q o", o=1)
                nc.vector.reciprocal(rcv[:, lo:hi + 1, :],
                                     av_q[:, lo:hi + 1, 64:65])
                for qs in fins:
                    nc.vector.tensor_scalar(
                        out=nrm[qs][:, side * 64:side * 64 + 64],
                        in0=av[side][:, qs * 128:qs * 128 + 64],
                        scalar1=rc[side][:, qs:qs + 1], scalar2=None,
                        op0=ALU.mult)
                    sdone[qs] = sdone.get(qs, 0) + 1
                    if sdone[qs] == 2:
                        qt = qc * 4 + qs
                        nc.sync.dma_start(
                            out=ctx4[pair][:, qt * 128:(qt + 1) * 128],
                            in_=nrm[qs], transpose=True)

            def emit_av(g, side, e):
                # one start=True per av bank: arming marks the whole 2KB
                # zero region pending, so later chains' first writes
                # (start=False) land fresh; a second start would re-arm
                # and wipe siblings' partials.
                for j in range(2):
                    kt = 2 * g + j
                    vb = (pair * TK + kt) * V2W + side * 66
                    for qs in range(max(0, kt - 4 * qc), 4):
                        nc.tensor.matmul(
                            av[side][:, qs * 128:qs * 128 + 65],
                            lhsT=e[:, j * 512 + qs * 128:
                                   j * 512 + (qs + 1) * 128],
                            rhs=v2all[:, vb:vb + 65],
                            start=((g, j) == arm and qs == 0),
                            stop=(last[qs] == (g, j)),
                            skip_group_check=True)
                fins = [qs for qs in range(4) if last[qs][0] == g]
                if fins:
                    finish(side, fins)

            for g in order:
                diag = g >= ngr - 2
                m = g - (ngr - 2)
                for side in range(2):
                    scp = scps.tile([128, 1024], f32, tag="sc", name="sc_ps")
                    for j in range(2):
                        kt = 2 * g + j
                        roff = min((2 * m + j) * 128, 256) if diag else 0
                        nc.tensor.matmul(
                            scp[:, j * 512 + roff:(j + 1) * 512],
                            lhsT=kT_v[pair][32 * side:32 * side + 32, :,
                                            kt * 128:(kt + 1) * 128],
                            rhs=qT_v[pair][32 * side:32 * side + 32, :,
                                           qc * 512 + roff:(qc + 1) * 512],
                            start=True, stop=True, perf_mode=DR)
                    e = esb.tile([128, 1024], bf16, tag="e", name="e_sb")
                    if diag and m == 1:
                        nc.scalar.activation(e[:, 256:512], scp[:, 256:512],
                                             AF.Exp, scale=LN2)
                        nc.scalar.activation(e[:, 896:1024], scp[:, 896:1024],
                                             AF.Exp, scale=LN2)
                    else:
                        xstate["n"] += 1
                        if K_XN and xstate["n"] % K_XN == 0:
                            dve_exp(e, scp)
                        else:
                            nc.scalar.activation(e, scp, AF.Exp, scale=LN2)
                    if diag:
                        for j in range(2):
                            r = 2 * m + j
                            c0 = j * 512 + r * 128
                            nc.gpsimd.tensor_mul(
                                e[:, c0:c0 + 128], e[:, c0:c0 + 128], tri_sb)
                    # lagged av batches stall on nothing (their exps are old),
                    # so emit them first; fillers follow (keeps finish-norms
                    # ahead of filler evacs in the DVE queue)
                    pend.append(lambda g=g, s=side, ee=e, f=emit_av: f(g, s, ee))
                    if len(pend) > K_LAG:
                        pend.pop(0)()
                    pump(force=force if side == 0 else 0, boost=boost)
                    if dbg and pair == 0 and qc == 0 and g == 0 and side == 0:
                        nc.sync.dma_start(out=edbg_d, in_=e)
            while len(pend) > K_XOVER:
                pend.pop(0)()

        def outproj(tt, oc, tail=False):
            # tail units alternate onto the (now idle) avps bank ring so two
            # units pipeline instead of serializing on one psum ring
            pool = avps if (tail and (tt + oc) % 2 == 0) else qkps
            tag = "av" if pool is avps else "p512"
            yp = pool.tile([128, 512], f32, tag=tag, name="y_ps")
            for f in range(4):
                nc.tensor.matmul(
                    yp, lhsT=ctx4[f][:, tt * 128:(tt + 1) * 128],
                    rhs=wo_sb[:, f * C + oc * 512: f * C + (oc + 1) * 512],
                    start=(f == 0), stop=(f == 3))
            ys = ystg.tile([128, 512], bf16, tag="ys", name="y_sb")
            if tail and (tt + oc) % 2 == 1:
                # ACT engine and its DGE queue are idle in the tail
                nc.scalar.activation(ys, yp, AF.Identity, scale=1.0)
                nc.scalar.dma_start(
                    out=y_d[tt * 128:(tt + 1) * 128, oc * 512:(oc + 1) * 512],
                    in_=ys)
            else:
                nc.vector.tensor_copy(ys, yp)
                nc.gpsimd.dma_start(
                    out=y_d[tt * 128:(tt + 1) * 128, oc * 512:(oc + 1) * 512],
                    in_=ys)

        # ---- emission schedule (program order = scheduler priority) ----
        if os.environ.get("K_P3D", "0") == "1":
            # pair 3 descending: outproj(qc3) releases early, program ends
            # on the smallest attention
            SEQ = [(p, qc) for p in range(3) for qc in range(4)] + \
                [(3, qc) for qc in (3, 2, 1, 0)]
        else:
            SEQ = [(p, qc) for p in range(4) for qc in range(4)]
        IDX = {pq: i for i, pq in enumerate(SEQ)}

        def unit(dl, cost, fn, nb=0):
            filler.append((dl, nb, cost, fn))

        def flush_until(idx):
            while filler and filler[0][0] <= idx:
                _, _, cost, fn = filler.pop(0)
                fn()
                fill_state["emitted"] += cost

        emit_qk(0, 0, 0, eng=nc.vector)
        emit_qk(1, 0, 0, eng=nc.vector)

        def qk_unit(tau, pair, c):
            unit(max(IDX[(pair, c)] - 1, 0), 854.0,
                 lambda t=tau, p=pair, cc=c: emit_qk(t, p, cc))

        def v_unit(kt, half):
            dl = IDX[(0 if half == 0 else 2, kt // 4)] - 1
            unit(max(dl, 1), 640.0, lambda k=kt, h=half: emit_v(k, h))

        for kt in range(4):
            v_unit(kt, 0)
        qk_unit(0, 0, 1)
        qk_unit(1, 0, 1)
        for kt in range(4, 8):
            v_unit(kt, 0)
        for c in range(2, 4):
            qk_unit(0, 0, c)
            qk_unit(1, 0, c)
            for kt in range(4 * c, 4 * c + 4):
                v_unit(kt, 0)
        for pair in range(1, 4):
            crange = ((3, 2, 1, 0) if pair == 3
                      and os.environ.get("K_P3D", "0") == "1" else range(4))
            for c in crange:
                qk_unit(0, pair, c)
                qk_unit(1, pair, c)
                if pair == 2:
                    for kt in range(4 * c, 4 * c + 4):
                        v_unit(kt, 1)

        for i, (pair, qc) in enumerate(SEQ):
            fill_state["cur"] = i
            flush_until(i)
            attn(pair, qc,
                 force=(K_F0 if i == 0 else (K_F123 if i <= 3 else 0)),
                 boost=K_BOOST if pair == 3 else 0.0)
            if pair == 3:
                for tt in range(4 * qc, 4 * qc + 4):
                    for oc in range(2):
                        unit(99, 853.0,
                             lambda t=tt, o=oc, tl=(i == 15): outproj(t, o, tl),
                             nb=fill_state["groups"] + K_NB)
        while pend:
            pend.pop(0)()
        while filler:
            _, _, cost, fn = filler.pop(0)
            fn()
        if dbg:
            nc.sync.dma_start(out=qdbg_d, in_=qT[0])
            nc.sync.dma_start(out=kdbg_d, in_=kT[0])
            nc.sync.dma_start(out=vdbg_d, in_=v2all)
            nc.sync.dma_start(out=cdbg_d, in_=ctx4[0])

    nc.compile()
    return nc


def _host_inputs(x, w_qkv, b_qkv, w_out):
    """Build the 8 per-core input maps (all partition-major layouts)."""
    f32 = np.float32
    tri = (np.arange(128)[:, None] <= np.arange(128)[None, :]).astype(BF)

    def split8(a):
        hi = a.astype(F8)
        lo = (a - hi.astype(f32)).astype(F8)
        return hi, lo

    def pmajor(a):
        w = a.shape[1]
        return np.ascontiguousarray(
            a.reshape(8, 128, w).transpose(1, 0, 2).reshape(128, 8 * w))

    def pmajor_x(a):
        # [C=(s p), T] -> [p, (chunk, s, t)]: 512-token chunks contiguous
        return np.ascontiguousarray(
            a.reshape(8, 128, 4, 512).transpose(1, 2, 0, 3).reshape(128, -1))

    xs = []
    for b in range(B):
        hi, lo = split8(np.ascontiguousarray(x[b].T))
        xs.append((pmajor_x(hi.astype(F8)), pmajor_x(lo.astype(F8))))

    p = np.arange(128)
    head_of_p = (p % 64) // 32
    feat_of_p = 32 * (p // 64) + (p % 32)

    in_maps = []
    for core in range(NCORES):
        b, hg = core // 2, core % 2
        cols = np.empty(1024, dtype=np.int64)
        for tau in range(2):
            for pair in range(4):
                base = (tau * 4 + pair) * 128
                cols[base:base + 128] = (tau * C + hg * FQ
                                         + (pair * 2 + head_of_p) * 64
                                         + feat_of_p)
        wqk = (np.ascontiguousarray(w_qkv[:, cols]) * 64.0).astype(F8)
        wq0 = pmajor(wqk[:, 0:128])
        wk0 = pmajor(wqk[:, 512:640])
        wrest = pmajor(np.concatenate(
            [wqk[:, 128:512], wqk[:, 640:1024]], axis=1))
        wv = w_qkv[:, 2 * C + hg * FQ: 2 * C + (hg + 1) * FQ] * 64.0
        wv_hi, wv_lo = split8(np.ascontiguousarray(wv))
        wv_hi, wv_lo = pmajor(wv_hi.astype(F8)), pmajor(wv_lo.astype(F8))

        po = np.arange(128)
        rows = np.empty(FQ, dtype=np.int64)
        for f in range(4):
            rows[f * 128:(f + 1) * 128] = (hg * FQ + (2 * f + po // 64) * 64
                                           + po % 64)
        wo16 = np.ascontiguousarray(
            w_out[rows, :].reshape(4, 128, C).transpose(1, 0, 2)
            .reshape(128, 4 * C)).astype(BF)

        # q bias laid out [64, (pair, slot)], pre-scaled by QA
        p64 = np.arange(64)
        bq64 = np.empty((64, 8), dtype=f32)
        for pair in range(4):
            for s in range(2):
                idx = hg * FQ + (pair * 2 + p64 // 32) * 64 + 32 * s + p64 % 32
                bq64[:, pair * 2 + s] = b_qkv[idx] * QA
        in_maps.append({
            "x8hi": xs[b][0], "x8lo": xs[b][1],
            "wq0": wq0, "wk0": wk0, "wrest": wrest,
            "wv8hi": wv_hi, "wv8lo": wv_lo,
            "wo16": wo16, "bq64": bq64, "tri16": tri,
        })
    return in_maps


def get_program():
    if "nc" not in _CACHE:
        _CACHE["nc"] = _build_program()
    return _CACHE["nc"]


def kernel(x, w_qkv, b_qkv, w_out, b_out):
    from concourse.bass_utils import run_bass_kernel_spmd

    x = np.asarray(x, dtype=np.float32)
    w_qkv = np.asarray(w_qkv, dtype=np.float32)
    b_qkv = np.asarray(b_qkv, dtype=np.float32)
    w_out = np.asarray(w_out, dtype=np.float32)
    b_out = np.asarray(b_out, dtype=np.float32)

    nc = get_program()
    in_maps = _host_inputs(x, w_qkv, b_qkv, w_out)
    res = run_bass_kernel_spmd(nc, in_maps, core_ids=list(range(NCORES)))

    bias = b_out + b_qkv[2 * C:] @ w_out        # folded v-bias
    out = np.empty((B, T, C), dtype=np.float32)
    for b in range(B):
        out[b] = (res.results[2 * b]["y"].astype(np.float32)
                  + res.results[2 * b + 1]["y"].astype(np.float32) + bias)
    return out
